# revision 7
# baseline (speedup 1.0000x reference)
"""Bahdanau additive attention on 8 TRN2 NeuronCores (data-parallel over batch).

reference math:
  q_proj = query @ W1 + b1                      # [B, U]
  v_proj = values @ W2 + b2                     # [B, T, U]
  score  = tanh(q_proj[:,None,:] + v_proj) @ Wv + bv   # [B, T, 1]
  aw     = softmax(score, axis=1)
  ctx    = sum(aw * values, axis=1)             # [B, D]
  returns (ctx, aw)

Sharding: batch B=32 split 4-per-core across 8 cores; W1/W2/Wv replicated.
Host pre-transposes values -> [b, D, T] and query -> [D, b] so the
contraction dim D lies on SBUF partitions (contiguous DMA, no on-chip
transpose).  bv is dropped: softmax is shift-invariant so it cancels in
both outputs.
"""

import numpy as np

B, T, D, U = 32, 2048, 1024, 1024
N_CORES = 8
BPC = B // N_CORES  # batches per core
P = 128
TTILE = 512
NTT = T // TTILE  # 4
NDB = D // P      # 8
NUB = U // P      # 8

MM_DT = "float32r"  # "float32r" (full-rate fp32-reduced) or "float32" (4x slower)

_cache = {}


def build_nc(mm_dt_name=MM_DT):
    """Build + compile the single-core Tile program (SPMD across 8 cores)."""
    from contextlib import ExitStack

    import concourse.bacc as bacc
    import concourse.mybir as mybir
    import concourse.tile as tile

    f32 = mybir.dt.float32
    # Tensors feeding FP32r matmuls must be *typed* float32r all the way from
    # their producer (walrus birverifier rule); np-view is float32 either way.
    mmdt = getattr(mybir.dt, mm_dt_name)
    AF = mybir.ActivationFunctionType
    ALU = mybir.AluOpType

    nc = bacc.Bacc("TRN2", target_bir_lowering=False)

    valuesT = nc.declare_dram_parameter("valuesT", [BPC, D, T], mmdt, isOutput=False)
    queryT = nc.declare_dram_parameter("queryT", [D, BPC], mmdt, isOutput=False)
    w1 = nc.declare_dram_parameter("W1", [D, U], mmdt, isOutput=False)
    w2 = nc.declare_dram_parameter("W2", [D, U], mmdt, isOutput=False)
    b1 = nc.declare_dram_parameter("b1", [U], f32, isOutput=False)
    b2 = nc.declare_dram_parameter("b2", [U], f32, isOutput=False)
    wv = nc.declare_dram_parameter("Wv", [U, 1], mmdt, isOutput=False)
    out_ctx = nc.declare_dram_parameter("out_ctx", [BPC, D], f32, isOutput=True)
    out_attn = nc.declare_dram_parameter("out_attn", [BPC, T], f32, isOutput=True)

    with ExitStack() as ctx:
        tc = ctx.enter_context(tile.TileContext(nc))
        singles = ctx.enter_context(tc.tile_pool(name="singles", bufs=1))
        vpool = ctx.enter_context(tc.tile_pool(name="vpool", bufs=3))
        thpool = ctx.enter_context(tc.tile_pool(name="thpool", bufs=4))
        epool = ctx.enter_context(tc.tile_pool(name="epool", bufs=2))
        opool = ctx.enter_context(tc.tile_pool(name="opool", bufs=2))
        ppv = ctx.enter_context(tc.tile_pool(name="ppv", bufs=2, space="PSUM"))
        pps = ctx.enter_context(tc.tile_pool(name="pps", bufs=2, space="PSUM"))
        ppb = ctx.enter_context(tc.tile_pool(name="ppb", bufs=2, space="PSUM"))
        ppq = ctx.enter_context(tc.tile_pool(name="ppq", bufs=1, space="PSUM"))

        # ---- stage 0: load weights, compute q_proj^T + b1 + b2 ----
        w1_sb = singles.tile([P, NDB, U], mmdt)
        nc.sync.dma_start(out=w1_sb, in_=w1.rearrange("(db p) u -> p db u", p=P))
        w2_sb = singles.tile([P, NDB, U], mmdt)
        nc.sync.dma_start(out=w2_sb, in_=w2.rearrange("(db p) u -> p db u", p=P))
        qT_sb = singles.tile([P, NDB, BPC], mmdt)
        nc.sync.dma_start(out=qT_sb, in_=queryT.rearrange("(db p) b -> p db b", p=P))
        b1_sb = singles.tile([P, NUB], f32)
        nc.sync.dma_start(out=b1_sb, in_=b1.rearrange("(ub p) -> p ub", p=P))
        b2_sb = singles.tile([P, NUB], f32)
        nc.sync.dma_start(out=b2_sb, in_=b2.rearrange("(ub p) -> p ub", p=P))
        wv_sb = singles.tile([P, NUB], mmdt)
        nc.sync.dma_start(out=wv_sb, in_=wv.rearrange("(ub p) one -> p (ub one)", p=P))
        ones_row = singles.tile([1, P], f32)
        nc.vector.memset(ones_row, 1.0)

        bsum_sb = singles.tile([P, NUB], f32)
        nc.vector.tensor_add(bsum_sb, b1_sb, b2_sb)

        qb_sb = singles.tile([P, NUB, BPC], f32)
        for ub in range(NUB):
            pq = ppq.tile([P, BPC], f32)
            for db in range(NDB):
                nc.tensor.matmul(
                    pq,
                    w1_sb[:, db, ub * P : (ub + 1) * P],
                    qT_sb[:, db, :],
                    start=(db == 0),
                    stop=(db == NDB - 1),
                )
            nc.vector.tensor_scalar_add(
                out=qb_sb[:, ub, :], in0=pq, scalar1=bsum_sb[:, ub : ub + 1]
            )

        # ---- main loop over batches and t-tiles ----
        for b in range(BPC):
            e_sb = epool.tile([1, T], f32, tag="e")
            z_sb = epool.tile([1, NTT], f32, tag="z")
            ctx_acc = opool.tile([P, NDB], f32, tag="ctx_acc")
            for tt in range(NTT):
                tsl = slice(tt * TTILE, (tt + 1) * TTILE)
                vt = vpool.tile([P, NDB, TTILE], mmdt)
                nc.sync.dma_start(
                    out=vt,
                    in_=valuesT[b, :, tsl].rearrange("(db p) t -> p db t", p=P),
                )
                ps = pps.tile([1, TTILE], f32)
                for ub in range(NUB):
                    pv = ppv.tile([P, TTILE], f32)
                    for db in range(NDB):
                        nc.tensor.matmul(
                            pv,
                            w2_sb[:, db, ub * P : (ub + 1) * P],
                            vt[:, db, :],
                            start=(db == 0),
                            stop=(db == NDB - 1),
                        )
                    th = thpool.tile([P, TTILE], mmdt)
                    nc.scalar.activation(
                        out=th, in_=pv, func=AF.Tanh, bias=qb_sb[:, ub, b : b + 1]
                    )
                    nc.tensor.matmul(
                        ps,
                        wv_sb[:, ub : ub + 1],
                        th,
                        start=(ub == 0),
                        stop=(ub == NUB - 1),
                    )
                # exp(score) with fused partial-sum for Z (softmax needs no
                # max-subtraction: |score| <= sum|Wv| ~ 26, safe in fp32)
                nc.scalar.activation(
                    out=e_sb[:, tsl],
                    in_=ps,
                    func=AF.Exp,
                    accum_out=z_sb[:, tt : tt + 1],
                )
                # broadcast e across partitions via K=1 ones-matmul
                pb = ppb.tile([P, TTILE], f32)
                nc.tensor.matmul(pb, ones_row, e_sb[:, tsl], start=True, stop=True)
                # ctx_acc[p, db] += sum_t vt[p, db, t] * e[t]
                cols = thpool.tile([P, NDB], f32, tag="cols")
                for db in range(NDB):
                    scr = thpool.tile([P, TTILE], f32, tag="scr")
                    nc.vector.tensor_mul(scr, vt[:, db, :].bitcast(f32), pb)
                    nc.vector.reduce_sum(
                        out=cols[:, db : db + 1], in_=scr, axis=mybir.AxisListType.X
                    )
                if tt == 0:
                    nc.vector.tensor_copy(ctx_acc, cols)
                else:
                    nc.vector.tensor_add(ctx_acc, ctx_acc, cols)
            # ---- per-batch epilogue: normalize ----
            zsum = opool.tile([1, 1], f32, tag="zsum")
            nc.vector.reduce_sum(out=zsum, in_=z_sb, axis=mybir.AxisListType.X)
            rz = opool.tile([1, 1], f32, tag="rz")
            nc.vector.reciprocal(out=rz, in_=zsum)
            aw = opool.tile([1, T], f32, tag="aw")
            nc.vector.tensor_scalar_mul(out=aw, in0=e_sb, scalar1=rz)
            nc.sync.dma_start(out=out_attn[b : b + 1, :], in_=aw)
            prz = ppq.tile([P, 1], f32, tag="prz")
            nc.tensor.matmul(prz, ones_row, rz, start=True, stop=True)
            rz128 = opool.tile([P, 1], f32, tag="rz128")
            nc.vector.tensor_copy(rz128, prz)
            ctxo = opool.tile([P, NDB], f32, tag="ctxo")
            nc.vector.tensor_scalar_mul(out=ctxo, in0=ctx_acc, scalar1=rz128)
            nc.sync.dma_start(
                out=out_ctx[b : b + 1, :].rearrange("one (db p) -> p (one db)", p=P),
                in_=ctxo,
            )

    nc.compile()
    return nc


def make_in_maps(query, values, W1, b1, W2, b2, Wv, bv):
    """Shard + pre-transpose host-side inputs for the 8 cores."""
    del bv  # shift-invariant under softmax; cancels in both outputs
    q = np.ascontiguousarray(query, dtype=np.float32)
    v = np.ascontiguousarray(values, dtype=np.float32)
    vT = np.ascontiguousarray(v.transpose(0, 2, 1))  # [B, D, T]
    W1 = np.ascontiguousarray(W1, dtype=np.float32)
    W2 = np.ascontiguousarray(W2, dtype=np.float32)
    b1 = np.ascontiguousarray(b1, dtype=np.float32)
    b2 = np.ascontiguousarray(b2, dtype=np.float32)
    Wv = np.ascontiguousarray(Wv, dtype=np.float32)
    in_maps = []
    for i in range(N_CORES):
        s = slice(i * BPC, (i + 1) * BPC)
        in_maps.append(
            {
                "valuesT": vT[s],
                "queryT": np.ascontiguousarray(q[s].T),
                "W1": W1,
                "W2": W2,
                "b1": b1,
                "b2": b2,
                "Wv": Wv,
            }
        )
    return in_maps


def kernel(query, values, W1, b1, W2, b2, Wv, bv):
    from concourse.bass_utils import run_bass_kernel_spmd

    if "nc" not in _cache:
        _cache["nc"] = build_nc()
    nc = _cache["nc"]
    in_maps = make_in_maps(query, values, W1, b1, W2, b2, Wv, bv)
    res = run_bass_kernel_spmd(nc, in_maps, core_ids=list(range(N_CORES)))
    ctx = np.concatenate([res.results[i]["out_ctx"] for i in range(N_CORES)], axis=0)
    aw = np.concatenate([res.results[i]["out_attn"] for i in range(N_CORES)], axis=0)
    return ctx, aw[:, :, None]


# revision 11
# speedup vs baseline: 1.0420x; 1.0420x over previous
"""Bahdanau additive attention on 8 TRN2 NeuronCores (data-parallel over batch).

reference math:
  q_proj = query @ W1 + b1                      # [B, U]
  v_proj = values @ W2 + b2                     # [B, T, U]
  score  = tanh(q_proj[:,None,:] + v_proj) @ Wv + bv   # [B, T, 1]
  aw     = softmax(score, axis=1)
  ctx    = sum(aw * values, axis=1)             # [B, D]
  returns (ctx, aw)

Sharding: batch B=32 split 4-per-core across 8 cores; W1/W2/Wv replicated.
Host pre-transposes values -> [b, D, T] and query -> [D, b] so the
contraction dim D lies on SBUF partitions (contiguous DMA, no on-chip
transpose).  bv is dropped: softmax is shift-invariant so it cancels in
both outputs.
"""

import numpy as np

B, T, D, U = 32, 2048, 1024, 1024
N_CORES = 8
BPC = B // N_CORES  # batches per core
P = 128
TTILE = 512
NTT = T // TTILE  # 4
NDB = D // P      # 8
NUB = U // P      # 8

MM_DT = "float32r"  # "float32r" (full-rate fp32-reduced) or "float32" (4x slower)

_cache = {}


def build_nc(mm_dt_name=MM_DT):
    """Build + compile the single-core Tile program (SPMD across 8 cores)."""
    from contextlib import ExitStack

    import concourse.bacc as bacc
    import concourse.mybir as mybir
    import concourse.tile as tile

    f32 = mybir.dt.float32
    # Tensors feeding FP32r matmuls must be *typed* float32r all the way from
    # their producer (walrus birverifier rule); np-view is float32 either way.
    mmdt = getattr(mybir.dt, mm_dt_name)
    AF = mybir.ActivationFunctionType
    ALU = mybir.AluOpType

    nc = bacc.Bacc("TRN2", target_bir_lowering=False)

    valuesT = nc.declare_dram_parameter("valuesT", [BPC, D, T], mmdt, isOutput=False)
    queryT = nc.declare_dram_parameter("queryT", [D, BPC], mmdt, isOutput=False)
    w1 = nc.declare_dram_parameter("W1", [D, U], mmdt, isOutput=False)
    w2 = nc.declare_dram_parameter("W2", [D, U], mmdt, isOutput=False)
    b1 = nc.declare_dram_parameter("b1", [U], f32, isOutput=False)
    b2 = nc.declare_dram_parameter("b2", [U], f32, isOutput=False)
    wv = nc.declare_dram_parameter("Wv", [U, 1], mmdt, isOutput=False)
    ones_d = nc.declare_dram_parameter("ones", [1, P], mmdt, isOutput=False)
    out_ctx = nc.declare_dram_parameter("out_ctx", [BPC, D], f32, isOutput=True)
    out_attn = nc.declare_dram_parameter("out_attn", [BPC, T], f32, isOutput=True)

    with ExitStack() as ctx:
        tc = ctx.enter_context(tile.TileContext(nc))
        singles = ctx.enter_context(tc.tile_pool(name="singles", bufs=1))
        vpool = ctx.enter_context(tc.tile_pool(name="vpool", bufs=3))
        thpool = ctx.enter_context(tc.tile_pool(name="thpool", bufs=4))
        epool = ctx.enter_context(tc.tile_pool(name="epool", bufs=2))
        opool = ctx.enter_context(tc.tile_pool(name="opool", bufs=2))
        ppv = ctx.enter_context(tc.tile_pool(name="ppv", bufs=2, space="PSUM"))
        pps = ctx.enter_context(tc.tile_pool(name="pps", bufs=2, space="PSUM"))
        ppb = ctx.enter_context(tc.tile_pool(name="ppb", bufs=2, space="PSUM"))
        ppq = ctx.enter_context(tc.tile_pool(name="ppq", bufs=1, space="PSUM"))

        # ---- stage 0: small tensors first (cheap, unblock q-proj chain) ----
        qT_sb = singles.tile([P, NDB, BPC], mmdt)
        nc.sync.dma_start(out=qT_sb, in_=queryT.rearrange("(db p) b -> p db b", p=P))
        b1_sb = singles.tile([P, NUB], f32)
        nc.sync.dma_start(out=b1_sb, in_=b1.rearrange("(ub p) -> p ub", p=P))
        b2_sb = singles.tile([P, NUB], f32)
        nc.sync.dma_start(out=b2_sb, in_=b2.rearrange("(ub p) -> p ub", p=P))
        wv_sb = singles.tile([P, NUB], mmdt)
        nc.sync.dma_start(out=wv_sb, in_=wv.rearrange("(ub p) one -> p (ub one)", p=P))
        ones_row = singles.tile([1, P], mmdt)
        nc.sync.dma_start(out=ones_row, in_=ones_d[:, :])

        bsum_sb = singles.tile([P, NUB], f32)
        nc.vector.tensor_add(bsum_sb, b1_sb, b2_sb)

        # W1/W2 loaded in u-block slices so the first tanh/score chain and the
        # first big matmuls unblock after ~0.5 MB instead of 8 MB of DMA.
        w1_sb = singles.tile([P, NDB, U], mmdt)
        w2_sb = singles.tile([P, NDB, U], mmdt)
        qb_sb = singles.tile([P, NUB, BPC], f32)
        w1_src = w1.rearrange("(db p) u -> p db u", p=P)
        w2_src = w2.rearrange("(db p) u -> p db u", p=P)

        vt_pre = {}

        def load_vt(b, tt):
            vt = vpool.tile([P, NDB, TTILE], mmdt, tag="vt", name=f"vt_{b}_{tt}")
            src = valuesT[b, :, tt * TTILE : (tt + 1) * TTILE].rearrange(
                "(db p) t -> p db t", p=P
            )
            for db in range(NDB):
                nc.sync.dma_start(out=vt[:, db, :], in_=src[:, db, :])
            return vt

        for ub in range(NUB):
            usl = slice(ub * P, (ub + 1) * P)
            nc.sync.dma_start(out=w2_sb[:, :, usl], in_=w2_src[:, :, usl])
            nc.sync.dma_start(out=w1_sb[:, :, usl], in_=w1_src[:, :, usl])
            if ub % 2 == 0:
                vt_pre[(0, ub // 2)] = load_vt(0, ub // 2)
            pq = ppq.tile([P, BPC], f32, tag="pq")
            for db in range(NDB):
                nc.tensor.matmul(
                    pq,
                    w1_sb[:, db, usl],
                    qT_sb[:, db, :],
                    start=(db == 0),
                    stop=(db == NDB - 1),
                )
            nc.vector.tensor_scalar_add(
                out=qb_sb[:, ub, :], in0=pq, scalar1=bsum_sb[:, ub : ub + 1]
            )

        # ---- main loop over batches and t-tiles ----
        for b in range(BPC):
            e_sb = epool.tile([1, T], mmdt, tag="e")
            z_sb = epool.tile([1, NTT], f32, tag="z")
            ctx_acc = opool.tile([P, NDB], f32, tag="ctx_acc")
            for tt in range(NTT):
                tsl = slice(tt * TTILE, (tt + 1) * TTILE)
                vt = vt_pre.pop((b, tt), None)
                if vt is None:
                    vt = load_vt(b, tt)
                ps = pps.tile([1, TTILE], f32)
                for ub in range(NUB):
                    pv = ppv.tile([P, TTILE], f32)
                    for db in range(NDB):
                        nc.tensor.matmul(
                            pv,
                            w2_sb[:, db, ub * P : (ub + 1) * P],
                            vt[:, db, :],
                            start=(db == 0),
                            stop=(db == NDB - 1),
                        )
                    th = thpool.tile([P, TTILE], mmdt)
                    nc.scalar.activation(
                        out=th, in_=pv, func=AF.Tanh, bias=qb_sb[:, ub, b : b + 1]
                    )
                    nc.tensor.matmul(
                        ps,
                        wv_sb[:, ub : ub + 1],
                        th,
                        start=(ub == 0),
                        stop=(ub == NUB - 1),
                    )
                # exp(score) with fused partial-sum for Z (softmax needs no
                # max-subtraction: |score| <= sum|Wv| ~ 26, safe in fp32)
                nc.scalar.activation(
                    out=e_sb[:, tsl],
                    in_=ps,
                    func=AF.Exp,
                    accum_out=z_sb[:, tt : tt + 1],
                )
                # broadcast e across partitions via K=1 ones-matmul
                pb = ppb.tile([P, TTILE], f32)
                nc.tensor.matmul(pb, ones_row, e_sb[:, tsl], start=True, stop=True)
                # ctx_acc[p, db] += sum_t vt[p, db, t] * e[t]
                cols = thpool.tile([P, NDB], f32, tag="cols")
                for db in range(NDB):
                    scr = thpool.tile([P, TTILE], f32, tag="scr")
                    nc.vector.tensor_mul(scr, vt[:, db, :].bitcast(f32), pb)
                    nc.vector.reduce_sum(
                        out=cols[:, db : db + 1], in_=scr, axis=mybir.AxisListType.X
                    )
                if tt == 0:
                    nc.vector.tensor_copy(ctx_acc, cols)
                else:
                    nc.vector.tensor_add(ctx_acc, ctx_acc, cols)
            # ---- per-batch epilogue: normalize ----
            zsum = opool.tile([1, 1], f32, tag="zsum")
            nc.vector.reduce_sum(out=zsum, in_=z_sb, axis=mybir.AxisListType.X)
            rz = opool.tile([1, 1], f32, tag="rz")
            nc.vector.reciprocal(out=rz, in_=zsum)
            aw = opool.tile([1, T], f32, tag="aw")
            nc.vector.tensor_scalar_mul(out=aw, in0=e_sb.bitcast(f32), scalar1=rz)
            nc.sync.dma_start(out=out_attn[b : b + 1, :], in_=aw)
            prz = ppq.tile([P, 1], f32, tag="prz")
            nc.tensor.matmul(prz, ones_row.bitcast(f32), rz, start=True, stop=True)
            rz128 = opool.tile([P, 1], f32, tag="rz128")
            nc.vector.tensor_copy(rz128, prz)
            ctxo = opool.tile([P, NDB], f32, tag="ctxo")
            nc.vector.tensor_scalar_mul(out=ctxo, in0=ctx_acc, scalar1=rz128)
            nc.sync.dma_start(
                out=out_ctx[b : b + 1, :].rearrange("one (db p) -> p (one db)", p=P),
                in_=ctxo,
            )

    nc.compile()
    return nc


def make_in_maps(query, values, W1, b1, W2, b2, Wv, bv):
    """Shard + pre-transpose host-side inputs for the 8 cores."""
    del bv  # shift-invariant under softmax; cancels in both outputs
    q = np.ascontiguousarray(query, dtype=np.float32)
    v = np.ascontiguousarray(values, dtype=np.float32)
    vT = np.ascontiguousarray(v.transpose(0, 2, 1))  # [B, D, T]
    W1 = np.ascontiguousarray(W1, dtype=np.float32)
    W2 = np.ascontiguousarray(W2, dtype=np.float32)
    b1 = np.ascontiguousarray(b1, dtype=np.float32)
    b2 = np.ascontiguousarray(b2, dtype=np.float32)
    Wv = np.ascontiguousarray(Wv, dtype=np.float32)
    in_maps = []
    for i in range(N_CORES):
        s = slice(i * BPC, (i + 1) * BPC)
        in_maps.append(
            {
                "valuesT": vT[s],
                "queryT": np.ascontiguousarray(q[s].T),
                "W1": W1,
                "W2": W2,
                "b1": b1,
                "b2": b2,
                "Wv": Wv,
                "ones": np.ones((1, 128), np.float32),
            }
        )
    return in_maps


def kernel(query, values, W1, b1, W2, b2, Wv, bv):
    from concourse.bass_utils import run_bass_kernel_spmd

    if "nc" not in _cache:
        _cache["nc"] = build_nc()
    nc = _cache["nc"]
    in_maps = make_in_maps(query, values, W1, b1, W2, b2, Wv, bv)
    res = run_bass_kernel_spmd(nc, in_maps, core_ids=list(range(N_CORES)))
    ctx = np.concatenate([res.results[i]["out_ctx"] for i in range(N_CORES)], axis=0)
    aw = np.concatenate([res.results[i]["out_attn"] for i in range(N_CORES)], axis=0)
    return ctx, aw[:, :, None]
